# revision 31
# baseline (speedup 1.0000x reference)
"""Trainium2 Bass kernel for nn_EnhancedAttentionLayer (segment softmax MLP).

Host contract: kernel(**inputs) takes the FULL unsharded inputs from
setup_inputs() and returns the FULL [N, 1] float32 output.

Strategy
--------
Math:  out = mean_heads( softmax_per_segment( sigmoid( Wa-head of
       LN( relu( relu([x, alpha] @ W1 + b1) @ W2 + b2 ) ) ) ) )

Device layout ("transposed chain", weights stationary as lhsT):
  xT [D, rows]  --W1-->  h1T [H, rows]  --W2-->  h2T [D, rows]
Matmul contracts over the partition dim, so feeding x transposed keeps
every matmul's moving operand at N=512 free-dim with zero on-device
transposes of activations in the main chain.

Both big matmuls run in fp8 e4m3 with MatmulPerfMode.DoubleRow (2 k-rows
per cycle, 2x bf16 throughput).  Scale folding keeps everything in the
fp8 normal range:  W1*SW1, b1*SW1 -> h1 carries SW1; W2*SW2 -> layer-2
PSUM carries SW1*SW2, removed by the relu activation's scale.

Folds (host-side precompute):
  - alpha column:  b1_eff = b1 + alpha * W1[D]   (concat removed)
  - LN scale/bias g,b folded into the head projection, and the rank-1
    -s_a*mu_r term folded into the weights:
      raw_att[r,a] = rstd[r]*p'[a,r] + c1[a]
      p' = wg'^T @ h2T,  wg'[d,a] = (g*Wa)[d,a] - s_a/D,
      s = sum_d g*Wa,  c1 = b@Wa + ba;  col 4 of wg' is 1/D -> mu.
  - segment max subtraction dropped: z = sigmoid(.) in (0,1), so
    softmax = exp(z)/segsum(exp(z)) is numerically safe without it.

Segment softmax on device: rows are pre-packed on host so every 512-row
supertile is segment-aligned (a segment never straddles supertiles) with
<=127 segments per supertile.  Per 128-row subtile, one-hot S[r, j] =
(j == local_seg[r]) and its transpose S^T[j, r] are both built with DVE
is_equal (fp16, no PE transposes); segment sums are S^T @ E (PE matmul,
fp16), the gather back is S @ recip(segsum) (PE matmul, fp16).

Sharding: 8 cores data-parallel over supertiles (segments never cross
cores).  Weights replicated.  SPMD: one Bass program, per-core inputs.
"""

import sys

sys.path.insert(0, "/opt/trn_rl_repo")

from contextlib import ExitStack

import ml_dtypes
import numpy as np

import concourse.bass as bass
import concourse.tile as tile
from concourse import bacc, bass_utils, mybir

BF16 = mybir.dt.bfloat16
E4 = mybir.dt.float8e4
F32 = mybir.dt.float32
AF = mybir.ActivationFunctionType
OP = mybir.AluOpType
PM = mybir.MatmulPerfMode
F16 = mybir.dt.float16

# fp8 scale folding: W1 is pre-scaled by SW1 and b1 by SW1, so h1 (sbuf, fp8)
# carries an extra SW1 factor; W2 is pre-scaled by SW2, so the layer-2 PSUM
# carries SW1*SW2, removed by the activation's scale parameter.
SW1 = 32.0
SW2 = 128.0

D = 512
H = 2048
HEADS = 4
RSUP = 512          # rows per supertile
NSUP = 25           # supertiles per core
NCORES = 8
EPS = 1e-5
KT1 = D // 128      # 4  k-tiles for layer 1
KT2 = H // 128      # 16 k-tiles for layer 2
HC1 = H // 128      # 16 output chunks for layer 1
DC2 = D // 128      # 4  output chunks for layer 2
NSUB = RSUP // 128  # 4  subtiles per supertile
PW = 33             # wg' columns: 4 heads + 28 zeros + 1/D (mu at 32)


def _build_bass(nsup: int):
    """Build the SPMD single-core Bass program for `nsup` supertiles.

    Two-deep software pipeline: during supertile st the kernel also emits
    the LN-stat finishing of st-1 (ms/var/transpose/rstd: "mid") and the
    sigmoid/softmax/normalize of st-2 ("tail"), so every cross-engine
    dependency is at least one supertile old and the PE never stalls.
    """
    nc = bacc.Bacc(
        "TRN2", target_bir_lowering=False, debug=False, enable_asserts=False
    )
    nr = nsup * RSUP

    xt_d = nc.dram_tensor("xt", [D, nr], E4, kind="ExternalInput").ap()
    cl_d = nc.dram_tensor("cl", [128, nsup * NSUB], F32, kind="ExternalInput").ap()
    cl2_d = nc.dram_tensor("cl2", [128, nr], BF16, kind="ExternalInput").ap()
    w1_d = nc.dram_tensor("w1", [D, H], E4, kind="ExternalInput").ap()
    b1_d = nc.dram_tensor("b1e", [128, HC1], F32, kind="ExternalInput").ap()
    w2_d = nc.dram_tensor("w2", [H, D], E4, kind="ExternalInput").ap()
    b2_d = nc.dram_tensor("b2t", [128, DC2], F32, kind="ExternalInput").ap()
    wg_d = nc.dram_tensor("wg", [D, PW], BF16, kind="ExternalInput").ap()
    e1_d = nc.dram_tensor("e1b", [128, NSUB * HEADS], F32, kind="ExternalInput").ap()
    io_d = nc.dram_tensor("iota", [128, 128], BF16, kind="ExternalInput").ap()
    iop_d = nc.dram_tensor("iotap", [128, 128], BF16, kind="ExternalInput").ap()
    id_d = nc.dram_tensor("ident", [64, 64], BF16, kind="ExternalInput").ap()
    out_d = nc.dram_tensor("out", [128, nsup * NSUB], F32, kind="ExternalOutput").ap()

    with tile.TileContext(nc) as tc, ExitStack() as ctx:
        consts = ctx.enter_context(tc.tile_pool(name="consts", bufs=1))
        xp = ctx.enter_context(tc.tile_pool(name="xp", bufs=2))
        hp = ctx.enter_context(tc.tile_pool(name="hp", bufs=2))
        ep = ctx.enter_context(tc.tile_pool(name="ep", bufs=3))
        sp = ctx.enter_context(tc.tile_pool(name="sp", bufs=3))
        pbig = ctx.enter_context(tc.tile_pool(name="pbig", bufs=3, space="PSUM"))
        pstat = ctx.enter_context(tc.tile_pool(name="pstat", bufs=2, space="PSUM"))
        pseg = ctx.enter_context(tc.tile_pool(name="pseg", bufs=1, space="PSUM"))
        psmall = ctx.enter_context(tc.tile_pool(name="psmall", bufs=2, space="PSUM"))

        # ---- constants, loaded once -------------------------------------
        # w1 + b1 + the first supertile's xT go first so layer 1 of
        # supertile 0 can start while the remaining consts stream in.
        xt_first = xp.tile([128, KT1, RSUP], E4, tag="xt")
        nc.sync.dma_start(
            out=xt_first,
            in_=xt_d[:, 0:RSUP].rearrange("(a p) c -> p a c", p=128),
        )
        b1_sb = consts.tile([128, HC1], F32)
        nc.sync.dma_start(out=b1_sb, in_=b1_d)
        w1_sb = consts.tile([128, KT1, H], E4)
        nc.sync.dma_start(out=w1_sb, in_=w1_d.rearrange("(a p) h -> p a h", p=128))
        w2_sb = consts.tile([128, KT2, D], E4)
        nc.sync.dma_start(out=w2_sb, in_=w2_d.rearrange("(a p) d -> p a d", p=128))
        wg_sb = consts.tile([128, KT1, PW], BF16)
        nc.sync.dma_start(out=wg_sb, in_=wg_d.rearrange("(a p) h -> p a h", p=128))
        b2_sb = consts.tile([128, DC2], F32)
        nc.sync.dma_start(out=b2_sb, in_=b2_d)
        e1_sb = consts.tile([128, NSUB * HEADS], F32)
        nc.sync.dma_start(out=e1_sb, in_=e1_d)
        iota_sb = consts.tile([128, 128], BF16)
        nc.sync.dma_start(out=iota_sb, in_=io_d)
        iop_sb = consts.tile([128, 128], BF16)
        nc.sync.dma_start(out=iop_sb, in_=iop_d)
        cl_sb = consts.tile([128, nsup * NSUB], F32)
        nc.sync.dma_start(out=cl_sb, in_=cl_d)

        ones_sb = consts.tile([128, 1], BF16)
        nc.vector.memset(ones_sb, 1.0 / D)
        id_sb = consts.tile([64, 64], BF16)
        nc.sync.dma_start(out=id_sb, in_=id_d)
        eps_sb = consts.tile([128, 1], F32)
        nc.vector.memset(eps_sb, EPS)

        out_sb = consts.tile([128, nsup * NSUB], F32)

        # -- pipeline pieces (emitted during later supertiles) ------------

        def tail_a(pl):
            """st-2: z = sigmoid(p'*rstd + c1) -> E = exp(z), batched over
            all 4 subtiles per activation op; then segment-sum matmuls."""
            t16 = sp.tile([128, NSUB * HEADS], F32, tag="t16", name="t16")
            for sub in range(NSUB):
                nc.vector.tensor_scalar(
                    t16[:, sub * HEADS : (sub + 1) * HEADS],
                    pl["pt_ev"][:, sub, 0:HEADS],
                    pl["rstd4"][:, sub : sub + 1], None, op0=OP.mult,
                )
            nc.vector.tensor_tensor(t16, t16, e1_sb, op=OP.add)
            sg16 = sp.tile([128, NSUB * HEADS], F32, tag="sg16", name="sg16")
            nc.scalar.activation(sg16, t16, AF.Sigmoid)
            e_t16 = sp.tile([128, NSUB * HEADS], F16, tag="e16", name="e_t16")
            nc.scalar.activation(e_t16, sg16, AF.Exp)
            seg_ps = pseg.tile([128, HEADS], F32, tag="seg", name="seg_ps")
            for sub in range(NSUB):
                nc.tensor.matmul(
                    seg_ps, pl["s_list"][sub],
                    e_t16[:, sub * HEADS : (sub + 1) * HEADS],
                    start=(sub == 0), stop=(sub == NSUB - 1),
                )
            pl["e_t16"], pl["seg_ps"] = e_t16, seg_ps

        def tail_d(pl):
            """st-2: 1/segsum, cast to fp16 for the gather matmul.  +1e-4
            keeps empty-slot reciprocals inside fp16 range (real segment
            sums are >= 1, so the perturbation is <= 1e-4 relative)."""
            segr = sp.tile([128, HEADS], F32, tag="segr", name="segr")
            nc.vector.tensor_scalar(segr, pl["seg_ps"], 1e-4, None, op0=OP.add)
            nc.vector.reciprocal(segr, segr)
            segr16 = sp.tile([128, HEADS], F16, tag="segr16", name="segr16")
            nc.vector.tensor_copy(segr16, segr)
            pl["segr16"] = segr16

        def mid_e(pl):
            """st-1: msq reduce, var into pv row 32, transpose to row-major,
            then rstd per row (ln+exp on [128,4], cheap)."""
            ms_ps = pl["p_ps"][64:65, :]
            nc.tensor.matmul(ms_ps, ones_sb, pl["sqs_sb"], start=True,
                             stop=True, skip_group_check=True)
            nc.vector.tensor_tensor(
                pl["pv_sb"][32:33, :], ms_ps, pl["mu2_sb"], op=OP.subtract
            )
            pv_ps = psmall.tile([128, NSUB, PW + 1], BF16, tag="ps_small")
            for sub in range(NSUB):
                nc.tensor.transpose(
                    pv_ps[:, sub, 0:PW],
                    pl["pv_sb"][:, sub * 128 : (sub + 1) * 128],
                    id_sb[0:PW, 0:PW],
                )
            pt_ev = sp.tile([128, NSUB, PW], F32, tag="pt_ev")
            nc.vector.tensor_copy(pt_ev, pv_ps[:, :, 0:PW])
            ln4 = sp.tile([128, NSUB], F32, tag="ln4", name="ln4")
            nc.scalar.activation(ln4, pt_ev[:, :, 32], AF.Ln, bias=eps_sb)
            rstd4 = sp.tile([128, NSUB], F32, tag="rstd4", name="rstd4")
            nc.scalar.activation(rstd4, ln4, AF.Exp, scale=-0.5)
            pl["pt_ev"], pl["rstd4"] = pt_ev, rstd4

        def tail_f(pl):
            """st-2: gather 1/segsum per row, normalize, reduce heads."""
            r_list = []
            for sub in range(NSUB):
                r_ps = psmall.tile([128, HEADS], F32, tag="ps_small", name="r_ps")
                nc.tensor.matmul(
                    r_ps, pl["st_t"][:, sub, :], pl["segr16"],
                    start=True, stop=True,
                )
                r_list.append(r_ps)
            for sub in range(NSUB):
                col = pl["st"] * NSUB + sub
                nrm = sp.tile([128, HEADS], F32, tag="nrm", name="nrm")
                nc.vector.tensor_tensor(
                    nrm, pl["e_t16"][:, sub * HEADS : (sub + 1) * HEADS],
                    r_list[sub], op=OP.mult,
                )
                nc.vector.tensor_reduce(
                    out_sb[:, col : col + 1], nrm,
                    axis=mybir.AxisListType.X, op=OP.add,
                )

        states: list = []
        for st in range(nsup):
            r0 = st * RSUP
            pl: dict = {"st": st}
            # ---- load x^T + broadcast seg-ids for this supertile --------
            if st == 0:
                xt_t = xt_first
            else:
                xt_t = xp.tile([128, KT1, RSUP], E4, tag="xt", name="xt_t")
                nc.sync.dma_start(
                    out=xt_t,
                    in_=xt_d[:, r0 : r0 + RSUP].rearrange("(a p) c -> p a c", p=128),
                )
            cl2_t = xp.tile([128, RSUP], BF16, tag="cl2", name="cl2_t")
            nc.sync.dma_start(out=cl2_t, in_=cl2_d[:, r0 : r0 + RSUP])

            # ---- one-hot S and S^T per subtile (DVE only, no PE) --------
            s_list = []
            st_t = ep.tile([128, NSUB, 128], F16, tag="st")
            for sub in range(NSUB):
                col = st * NSUB + sub
                s_sb = sp.tile([128, 128], F16, tag="s_sb", bufs=12, name="s_sb")
                nc.vector.tensor_scalar(
                    s_sb, iota_sb, cl_sb[:, col : col + 1], None, op0=OP.is_equal
                )
                s_list.append(s_sb)
                nc.vector.tensor_tensor(
                    st_t[:, sub, :], iop_sb,
                    cl2_t[:, sub * 128 : (sub + 1) * 128], op=OP.is_equal,
                )
            pl["s_list"], pl["st_t"] = s_list, st_t

            # ---- layer 1: h1T[hc] = relu(W1[:,hc].T @ xT + b1) ----------
            h1_t = hp.tile([128, HC1, RSUP], E4, tag="h1")
            for hc in range(HC1):
                ps1 = pbig.tile([128, RSUP], F32, tag="pbig")
                for kd in range(KT1 // 2):
                    nc.tensor.matmul(
                        ps1,
                        w1_sb[:, 2 * kd : 2 * kd + 2, hc * 128 : (hc + 1) * 128],
                        xt_t[:, 2 * kd : 2 * kd + 2, :],
                        start=(kd == 0),
                        stop=(kd == KT1 // 2 - 1),
                        perf_mode=PM.DoubleRow,
                    )
                if hc % 8 < 5:
                    nc.scalar.activation(
                        h1_t[:, hc, :], ps1, AF.Relu, bias=b1_sb[:, hc : hc + 1]
                    )
                else:
                    nc.vector.tensor_scalar(
                        h1_t[:, hc, :], ps1, b1_sb[:, hc : hc + 1], 0.0,
                        op0=OP.add, op1=OP.max,
                    )

            # ---- layer 2 (+ head projection, one dc late) ---------------
            h2_t = hp.tile([128, DC2, RSUP], BF16, tag="h2")
            sq_t = hp.tile([128, DC2, RSUP], BF16, tag="sq")
            p_ps = pstat.tile([65, RSUP], F32, tag="p")
            pl["p_ps"] = p_ps

            def head_mm(dc):
                nc.tensor.matmul(
                    p_ps[0:PW, :], wg_sb[:, dc, :], h2_t[:, dc, :],
                    start=(dc == 0), stop=(dc == DC2 - 1),
                )

            for dc in range(DC2):
                ps2 = pbig.tile([128, RSUP], F32, tag="pbig")
                for kd in range(KT2 // 2):
                    nc.tensor.matmul(
                        ps2,
                        w2_sb[:, 2 * kd : 2 * kd + 2, dc * 128 : (dc + 1) * 128],
                        h1_t[:, 2 * kd : 2 * kd + 2, :],
                        start=(kd == 0),
                        stop=(kd == KT2 // 2 - 1),
                        perf_mode=PM.DoubleRow,
                    )
                h2c = h2_t[:, dc, :]
                nc.scalar.activation(
                    h2c, ps2, AF.Relu,
                    bias=b2_sb[:, dc : dc + 1], scale=1.0 / (SW1 * SW2),
                )
                nc.gpsimd.tensor_tensor(
                    sq_t[:, dc, :], h2c, h2c, op=OP.mult
                )
                if dc >= 1:
                    head_mm(dc - 1)
            head_mm(DC2 - 1)

            # msq partials: tree-sum the squares on the (otherwise idle)
            # gpsimd engine; the final reduce matmul runs one supertile
            # later (mid_e) so its 8us latency is off the critical path.
            t01_sb = hp.tile([128, RSUP], BF16, tag="t01")
            nc.gpsimd.tensor_tensor(t01_sb, sq_t[:, 0, :], sq_t[:, 1, :], op=OP.add)
            t23_sb = hp.tile([128, RSUP], BF16, tag="t23")
            nc.gpsimd.tensor_tensor(t23_sb, sq_t[:, 2, :], sq_t[:, 3, :], op=OP.add)
            sqs_sb = hp.tile([128, RSUP], BF16, tag="sqs")
            nc.gpsimd.tensor_tensor(sqs_sb, t01_sb, t23_sb, op=OP.add)
            pl["sqs_sb"] = sqs_sb

            # st-2: sigmoid/E/segment-sum (PE seg matmuls land after heads)
            if st >= 2:
                tail_a(states[st - 2])

            # ---- evict stats from PSUM ----------------------------------
            # pv rows 0..3 = -p' (heads; mu and sign folded on host), rows
            # 4..31 zero, row 32 = mu, overwritten by var in mid_e.
            mu_sb = sp.tile([1, RSUP], F32, tag="mu_sb")
            nc.vector.tensor_copy(mu_sb, p_ps[32:33, :])
            mu2_sb = sp.tile([1, RSUP], F32, tag="mu2_sb")
            nc.vector.tensor_tensor(mu2_sb, mu_sb, mu_sb, op=OP.mult)
            pv_sb = sp.tile([PW, RSUP], BF16, tag="pv_sb")
            nc.vector.tensor_copy(pv_sb, p_ps[0:PW, :])
            pl["mu2_sb"], pl["pv_sb"] = mu2_sb, pv_sb

            if st >= 2:
                tail_d(states[st - 2])
            if st >= 1:
                mid_e(states[st - 1])
            if st >= 2:
                tail_f(states[st - 2])
            states.append(pl)

        # drain the pipeline (interleave the two remaining chains)
        mid_e(states[nsup - 1])
        tail_a(states[nsup - 2])
        tail_a(states[nsup - 1])
        tail_d(states[nsup - 2])
        tail_d(states[nsup - 1])
        tail_f(states[nsup - 2])
        tail_f(states[nsup - 1])
        nc.sync.dma_start(out=out_d, in_=out_sb)

    nc.finalize()
    return nc


_BUILD_CACHE: dict = {}


def _get_bass(nsup: int):
    if nsup not in _BUILD_CACHE:
        _BUILD_CACHE[nsup] = _build_bass(nsup)
    return _BUILD_CACHE[nsup]


def _host_prep(x, row, alpha, W1, b1, W2, b2, ln_g, ln_b, Wa, ba,
               nsup=NSUP, ncores=NCORES):
    """Pack rows into segment-aligned supertiles, build per-core inputs."""
    bf16 = ml_dtypes.bfloat16
    e4 = ml_dtypes.float8_e4m3
    N = x.shape[0]
    row = np.asarray(row).astype(np.int64)

    # segment runs (row is sorted)
    change = np.flatnonzero(np.diff(row)) + 1
    starts = np.concatenate([[0], change])
    ends = np.concatenate([change, [N]])
    lens = ends - starts
    assert lens.max() <= RSUP, "segment longer than a supertile"

    # greedy pack segments into RSUP-row bins
    bin_of_seg = np.empty(len(starts), np.int64)
    nbins = 0
    cur = 0
    for i, ln in enumerate(lens):
        if cur + ln > RSUP:
            nbins += 1
            cur = 0
        bin_of_seg[i] = nbins
        cur += ln
    nbins += 1
    assert nbins <= ncores * nsup, f"{nbins} bins > capacity {ncores * nsup}"

    nr = nsup * RSUP
    # per-core gather index (source row or -1) and local segment rank
    gidx = np.full((ncores, nr), -1, np.int64)
    cloc = np.full((ncores, nr), 127.0, np.float32)
    seg_rank = np.zeros(len(starts), np.int64)
    # rank of each segment within its bin; position of each segment in bin
    pos_in_bin = np.zeros(len(starts), np.int64)
    cur_bin, cur_pos, cur_rank = -1, 0, 0
    for i in range(len(starts)):
        if bin_of_seg[i] != cur_bin:
            cur_bin, cur_pos, cur_rank = bin_of_seg[i], 0, 0
        pos_in_bin[i] = cur_pos
        seg_rank[i] = cur_rank
        cur_pos += lens[i]
        cur_rank += 1
    assert seg_rank.max() <= 126, "too many segments in one supertile"

    for i in range(len(starts)):
        b = bin_of_seg[i]
        c, stl = divmod(b, nsup)
        base = stl * RSUP + pos_in_bin[i]
        gidx[c, base : base + lens[i]] = np.arange(starts[i], ends[i])
        cloc[c, base : base + lens[i]] = seg_rank[i]

    # x gather + transpose + fp8 e4m3, per core
    x_ext = np.concatenate([x, np.zeros((1, D), np.float32)], 0).astype(e4)
    srcs = np.where(gidx < 0, N, gidx)
    xts = []
    for c in range(ncores):
        xc = x_ext[srcs[c]]                       # [nr, D] fp8
        xts.append(np.ascontiguousarray(xc.T))    # [D, nr] fp8

    # cl layout [128, nsup*NSUB]: cl2[p, st*NSUB+sub] = cloc[st*512+sub*128+p]
    cls = [
        np.ascontiguousarray(
            cloc[c].reshape(nsup * NSUB, 128).T
        ).astype(np.float32)
        for c in range(ncores)
    ]
    # cl broadcast down partitions for the direct S^T build
    cl2s = [
        np.ascontiguousarray(
            np.broadcast_to(cloc[c].astype(bf16), (128, nr))
        )
        for c in range(ncores)
    ]

    # replicated weights / constants (W1/W2 pre-scaled into fp8 normal range)
    alpha_f = float(np.asarray(alpha).reshape(-1)[0])
    w1 = np.ascontiguousarray(W1[:D] * SW1).astype(e4)                # [D, H]
    b1_eff = ((b1 + alpha_f * W1[D]) * SW1).astype(np.float32)        # [H]
    b1e = np.ascontiguousarray(b1_eff.reshape(HC1, 128).T)            # [128,16]
    w2 = (W2 * SW2).astype(e4)                                        # [H, D]
    b2t = np.ascontiguousarray(b2.astype(np.float32).reshape(DC2, 128).T)
    wg_f = (ln_g[:, None] * Wa).astype(np.float32)                    # [D, 4]
    s_vec = wg_f.sum(0).astype(np.float32)                            # [4]
    # rank-1 -s*mu fold; device computes sigmoid(p'*rstd + c1) directly
    wgp = wg_f - s_vec[None, :] / D
    wg = np.concatenate(
        [wgp, np.zeros((D, 28), np.float32),
         np.full((D, 1), 1.0 / D, np.float32)], 1
    ).astype(bf16)                                                    # [D, 33]
    c1 = (ln_b @ Wa + ba).astype(np.float32)                          # [4]
    e1b = np.broadcast_to(np.tile(c1, NSUB), (128, NSUB * HEADS)).astype(np.float32).copy()
    iota = np.broadcast_to(
        np.arange(128, dtype=np.float32), (128, 128)
    ).astype(bf16).copy()
    iotap = np.ascontiguousarray(iota.T)
    ident = np.eye(64, dtype=np.float32).astype(bf16)

    in_maps = []
    for c in range(ncores):
        in_maps.append({
            "xt": xts[c], "cl": cls[c], "cl2": cl2s[c], "w1": w1, "b1e": b1e,
            "w2": w2, "b2t": b2t, "wg": wg, "e1b": e1b, "iota": iota,
            "iotap": iotap, "ident": ident,
        })
    return in_maps, gidx


def _unshard(results, gidx, N):
    out = np.zeros((N, 1), np.float32)
    for c, res in enumerate(results):
        vals = np.ascontiguousarray(res["out"].T).reshape(-1, NSUB, 128)
        vals = vals.reshape(-1)  # position order (st, sub, p)
        m = gidx[c] >= 0
        out[gidx[c][m], 0] = vals[m]
    return out * (1.0 / HEADS)


def kernel(x, row, alpha, W1, b1, W2, b2, ln_g, ln_b, Wa, ba, **_kw):
    x = np.asarray(x, np.float32)
    in_maps, gidx = _host_prep(
        x, row, alpha,
        np.asarray(W1, np.float32), np.asarray(b1, np.float32),
        np.asarray(W2, np.float32), np.asarray(b2, np.float32),
        np.asarray(ln_g, np.float32), np.asarray(ln_b, np.float32),
        np.asarray(Wa, np.float32), np.asarray(ba, np.float32),
    )
    nc = _get_bass(NSUP)
    res = bass_utils.run_bass_kernel_spmd(
        nc, in_maps, core_ids=list(range(NCORES))
    )
    return _unshard(res.results, gidx, x.shape[0])
